# revision 25
# baseline (speedup 1.0000x reference)
"""Trainium2 Bass kernel for nn_ConcatSquashLinearSA.

Math (per sample b, S=1):
    gate = sigmoid(ctx @ Wg.T + bg)          [256]
    bias = ctx @ Wb.T                        [256]
    kv   = ctx @ Wkv.T                       [256]
    E    = outer(kv, kv)                     [256,256]
    A    = softmax_rows(E)
    att  = A / (1e-9 + colsum(A))
    out  = (x @ Wl.T + bl) @ (I + att) * gate + bias

which folds to a single big matmul per sample:
    P'     = A / colsum(A)                  (gate factored out)
    W_raw  = Wl.T @ P' + Wl.T               [256,256]  (tiny, on-device, f32)
    b_fin  = (bl + bl @ P') * gate + bias   [256]
    out    = (x @ W_raw) * gate + b_fin     [16384,256] (the only big op)

Sharding: data-parallel over batch, 2 samples per core across 8 cores.

The big op is memory-bound, so all big I/O is bf16 (2e-2 tolerance; bf16
end-to-end emulated rel-err is 2.7e-3). The host pre-transposes x to
x^T [256, rows] bf16; the device computes out^T = (W_raw^T @ x^T) * gate
+ b_fin with e on partitions (gate/b_fin are per-partition scalars of the
PSUM->SBUF copy), and the host transposes back. This removes all
on-device transposes and halves HBM traffic vs f32.

Schedule notes (from perfetto/NTFF traces):
  - every dma_start costs ~0.65us of issuing-engine time: constants are
    packed into 2 DMAs, reads all go on the otherwise-idle sync ring
  - softmax/W_raw preamble is a serial cross-engine chain (~1us/hop):
    exp+rowsum fused via accum_out, colsum taken directly from expE with
    lhsT=rowrecip, gate applied at copy time (not in the preamble)
  - reads run ~12 chunks ahead (xin bufs) so HBM streams during preamble
  - psum tiles span 2 banks -> half the copies/semaphores; copies split
    3:1 between DVE and ACT (gpsimd has no PSUM port)
"""

import numpy as np

B, N, DIN, DOUT, DCTX = 16, 16384, 256, 256, 131
NCORES = 8
SPC = B // NCORES           # samples per core
ROWS = SPC * N              # x rows per core (columns of x^T)
CH = 2048                   # x^T columns per macro-chunk

# column offsets in the packed constant blocks (hot: needed by first MMs)
_PK_CTXT0, _PK_CTXT1 = 0, 2
_PK_WCAT0, _PK_WCAT1 = 4, 772
_HOT_COLS = 1540
_PK_WL0, _PK_WL1 = 0, 256
_PK_WLT0, _PK_WLT1 = 512, 768
_PK_BLC0, _PK_BLC1, _PK_ONESC = 1024, 1025, 1026
_COLD_COLS = 1027
# row pack [1, 1024]: blr | bgr | ones512
_RP_BLR, _RP_BGR, _RP_ONES = 0, 256, 512


def build_nc(rows=ROWS):
    import concourse.bass as bass
    import concourse.tile as tile
    from concourse import bacc, mybir
    from contextlib import ExitStack

    f32 = mybir.dt.float32
    bf16 = mybir.dt.bfloat16
    AF = mybir.ActivationFunctionType
    AX = mybir.AxisListType
    OP = mybir.AluOpType

    n_chunks = rows // CH

    nc = bacc.Bacc()
    x_d = nc.declare_dram_parameter("xT", [128, 2 * rows], bf16, isOutput=False)
    hot_d = nc.declare_dram_parameter("packhot", [128, _HOT_COLS], f32,
                                      isOutput=False)
    cold_d = nc.declare_dram_parameter("packcold", [128, _COLD_COLS], f32,
                                       isOutput=False)
    rowp_d = nc.declare_dram_parameter("rowpack", [1, 1024], f32,
                                       isOutput=False)
    o0_d = nc.declare_dram_parameter("outT0", [128, rows], bf16, isOutput=True)
    o1_d = nc.declare_dram_parameter("outT1", [128, rows], bf16, isOutput=True)

    with tile.TileContext(nc) as tc, ExitStack() as ctx:
        consts = ctx.enter_context(tc.tile_pool(name="consts", bufs=1))
        spool = ctx.enter_context(tc.tile_pool(name="scratch", bufs=2))
        perm = ctx.enter_context(tc.tile_pool(name="persample", bufs=1))
        pps = ctx.enter_context(tc.tile_pool(name="pps", bufs=1, space="PSUM"))
        pout = ctx.enter_context(tc.tile_pool(name="pout", bufs=7, space="PSUM"))
        xin = ctx.enter_context(tc.tile_pool(name="xin", bufs=12))
        osb = ctx.enter_context(tc.tile_pool(name="osb", bufs=4))
        xlast = ctx.enter_context(tc.tile_pool(name="xlast", bufs=1))
        olast = ctx.enter_context(tc.tile_pool(name="olast", bufs=1))

        hot = consts.tile([128, _HOT_COLS], f32, name="packhot", tag="packhot")
        nc.sync.dma_start(hot, hot_d[:, :])
        rp = consts.tile([1, 1024], f32, name="rowpack", tag="rowpack")
        nc.sync.dma_start(rp, rowp_d[:, :])
        cold = consts.tile([128, _COLD_COLS], f32, name="packcold",
                           tag="packcold")
        nc.sync.dma_start(cold, cold_d[:, :])

        ctxT = [hot[:, _PK_CTXT0:_PK_CTXT0 + SPC], hot[:, _PK_CTXT1:_PK_CTXT1 + SPC]]
        wcat = [hot[:, _PK_WCAT0:_PK_WCAT0 + 768], hot[:, _PK_WCAT1:_PK_WCAT1 + 768]]
        wl = [cold[:, _PK_WL0:_PK_WL0 + 256], cold[:, _PK_WL1:_PK_WL1 + 256]]
        wlT = [cold[:, _PK_WLT0:_PK_WLT0 + 256], cold[:, _PK_WLT1:_PK_WLT1 + 256]]
        blc = [cold[:, _PK_BLC0:_PK_BLC0 + 1], cold[:, _PK_BLC1:_PK_BLC1 + 1]]
        onesc = cold[:, _PK_ONESC:_PK_ONESC + 1]
        blr = rp[0:1, _RP_BLR:_RP_BLR + 256]
        bgr = rp[0:1, _RP_BGR:_RP_BGR + 256]
        onesr = rp[0:1, _RP_ONES:_RP_ONES + 128]
        ones1 = rp[0:1, _RP_ONES:_RP_ONES + 1]

        # warm the ACT/DVE function tables before the real chain needs them
        warm = spool.tile([1, 4], f32, name="warm", tag="warm")
        nc.scalar.activation(warm[0:1, 0:1], ones1, AF.Exp)
        nc.scalar.activation(warm[0:1, 1:2], ones1, AF.Tanh)
        nc.scalar.activation(warm[0:1, 2:3], ones1, AF.Identity)
        nc.vector.reciprocal(warm[0:1, 3:4], ones1)

        weff = {}
        ccol = {}   # (s, j) -> [128,1] gate^T half (per-partition out scale)
        bcol = {}   # (s, j) -> [128,1] b_fin^T half (per-partition out bias)
        for s in range(SPC):
            # ---- ctx projections: [gate_pre | bias | kv] = ctx @ WcatT ----
            cat1 = pps.tile([1, 512], f32, name=f"cat1_{s}", tag="ps")
            nc.tensor.matmul(cat1, lhsT=ctxT[0][:, s:s + 1],
                             rhs=wcat[0][:, 0:512], start=True, stop=False)
            nc.tensor.matmul(cat1, lhsT=ctxT[1][:, s:s + 1],
                             rhs=wcat[1][:, 0:512], start=False, stop=True)
            cat2 = pps.tile([1, 256], f32, name=f"cat2_{s}", tag="ps")
            nc.tensor.matmul(cat2, lhsT=ctxT[0][:, s:s + 1],
                             rhs=wcat[0][:, 512:768], start=True, stop=False)
            nc.tensor.matmul(cat2, lhsT=ctxT[1][:, s:s + 1],
                             rhs=wcat[1][:, 512:768], start=False, stop=True)
            svec = spool.tile([1, 768], f32, name=f"svec{s}", tag="svec")
            nc.vector.tensor_copy(svec[:, 0:512], cat1)
            nc.vector.tensor_copy(svec[:, 512:768], cat2)

            # gate = sigmoid(pre) = 0.5*tanh(0.5*pre) + 0.5  (same ACT table)
            gpre = spool.tile([1, 256], f32, name=f"gpre{s}", tag="gpre")
            nc.vector.tensor_add(gpre, svec[:, 0:256], bgr)
            th = spool.tile([1, 256], f32, name=f"th{s}", tag="th")
            nc.scalar.activation(th, gpre, AF.Tanh, scale=0.5)
            gate = spool.tile([1, 256], f32, name=f"gate{s}", tag="gate")
            nc.vector.tensor_scalar(gate, th, 0.5, 0.5, op0=OP.mult, op1=OP.add)

            # ---- E = outer(kv, kv); fused exp+rowsum (|E|<=~8: exp safe) ----
            expEs, rcs_ = [], []
            for i in range(2):
                E = pps.tile([128, 256], f32, name=f"E{s}{i}", tag="ps")
                nc.tensor.matmul(E, lhsT=svec[0:1, 512 + 128 * i:640 + 128 * i],
                                 rhs=svec[0:1, 512:768], start=True, stop=True)
                expE = spool.tile([128, 256], f32, name=f"expE{s}{i}", tag="expE")
                sm = spool.tile([128, 1], f32, name=f"sm{s}{i}", tag="sm")
                nc.scalar.activation(expE, E, AF.Exp, accum_out=sm)
                rc = spool.tile([128, 1], f32, name=f"rc{s}{i}", tag="rc")
                nc.vector.reciprocal(rc, sm)
                expEs.append(expE)
                rcs_.append(rc)

            # colsum(A) directly from expE: cs_e = sum_d rc[d]*expE[d,e]
            cs = pps.tile([1, 256], f32, name=f"cs{s}", tag="ps")
            nc.tensor.matmul(cs, lhsT=rcs_[0], rhs=expEs[0], start=True, stop=False)
            nc.tensor.matmul(cs, lhsT=rcs_[1], rhs=expEs[1], start=False, stop=True)
            rcsum = spool.tile([1, 256], f32, name=f"rcsum{s}", tag="rcsum")
            nc.vector.reciprocal(rcsum, cs)   # colsum >= 0.8 on this data

            # broadcast 1/colsum to [128,256]; P' = expE * rc (row) * (col)
            vbp = pps.tile([128, 256], f32, name=f"vbp{s}", tag="ps")
            nc.tensor.matmul(vbp, lhsT=onesr, rhs=rcsum, start=True, stop=True)
            Vb = spool.tile([128, 256], f32, name=f"Vb{s}", tag="Vb")
            nc.vector.tensor_copy(Vb, vbp)
            P = []
            for i in range(2):
                P1 = spool.tile([128, 256], f32, name=f"P1{s}{i}", tag="P1")
                nc.vector.tensor_scalar_mul(P1, expEs[i], rcs_[i])
                Pi = spool.tile([128, 256], f32, name=f"P{s}{i}", tag="P")
                nc.vector.tensor_mul(Pi, P1, Vb)
                P.append(Pi)

            # ---- W_raw = Wl.T @ P' + Wl.T; rows d-half i, bf16 ----
            for i in range(2):
                wp = pps.tile([128, 256], f32, name=f"wp{s}{i}", tag="ps")
                nc.tensor.matmul(wp, lhsT=wl[0][:, 128 * i:128 * (i + 1)], rhs=P[0],
                                 start=True, stop=False)
                nc.tensor.matmul(wp, lhsT=wl[1][:, 128 * i:128 * (i + 1)], rhs=P[1],
                                 start=False, stop=True)
                wsb = perm.tile([128, 256], bf16, name=f"weff{s}{i}",
                                tag=f"weff{s}{i}")
                nc.vector.tensor_add(wsb, wp, wlT[i])
                weff[(s, i)] = wsb

            # ---- b_fin = (bl + bl @ P') * gate + bias ----
            qp = pps.tile([1, 256], f32, name=f"qp{s}", tag="ps")
            nc.tensor.matmul(qp, lhsT=blc[0], rhs=P[0], start=True, stop=False)
            nc.tensor.matmul(qp, lhsT=blc[1], rhs=P[1], start=False, stop=True)
            tb2 = spool.tile([1, 256], f32, name=f"tb2{s}", tag="tb2")
            nc.vector.tensor_add(tb2, blr, qp)
            tb3 = spool.tile([1, 256], f32, name=f"tb3{s}", tag="tb3")
            nc.vector.tensor_mul(tb3, tb2, gate)
            bfin = spool.tile([1, 256], f32, name=f"bfin{s}", tag="bfin")
            nc.vector.tensor_add(bfin, tb3, svec[:, 256:512])

            # ---- per-partition copy scalars: gate^T and b_fin^T halves ----
            for j in range(2):
                cbp = pps.tile([128, 2], f32, name=f"cbp{s}{j}", tag="ps")
                nc.tensor.matmul(cbp[:, 0:1], lhsT=gate[0:1, 128 * j:128 * (j + 1)],
                                 rhs=ones1, start=True, stop=True)
                nc.tensor.matmul(cbp[:, 1:2], lhsT=bfin[0:1, 128 * j:128 * (j + 1)],
                                 rhs=ones1, start=True, stop=True)
                cb = perm.tile([128, 2], f32, name=f"cb{s}{j}", tag=f"cb{s}{j}")
                nc.vector.tensor_copy(cb, cbp)
                ccol[(s, j)] = cb[:, 0:1]
                bcol[(s, j)] = cb[:, 1:2]

        # ---- main loop: out^T[e,n] = (sum_d W_raw[d,e] x^T[d,n])*gate[e]
        #      + b_fin[e];  e-half j on partitions, d contracted; bf16 streams.
        x3 = x_d.rearrange("p (i n) -> p i n", i=2)
        sched = [(CH * t, CH, "") for t in range(n_chunks - 1)]
        base = CH * (n_chunks - 1)
        sched += [(base, 1024, "a"), (base + 1024, 512, "b"),
                  (base + 1536, 512, "c")]
        for (c0, w, sfx) in sched:
            s = c0 // (rows // SPC)
            xpool, opool = (xin, osb) if not sfx else (xlast, olast)
            xt = xpool.tile([128, 2, w], bf16, name="xt" + sfx, tag="xt" + sfx)
            nc.sync.dma_start(xt, x3[:, :, c0:c0 + w])
            ot = opool.tile([128, 2 * w], bf16, name="ot" + sfx, tag="ot" + sfx)
            nsl = w // 512
            ci = 0
            for j in range(2):
                # one stationary weight per (i, j): nsl consecutive matmuls
                pss = [pout.tile([128, 512], f32, name="ps", tag="ops")
                       for _ in range(nsl)]
                for i in range(2):
                    for h in range(nsl):
                        nc.tensor.matmul(
                            pss[h],
                            lhsT=weff[(s, i)][:, 128 * j:128 * (j + 1)],
                            rhs=xt[:, i, 512 * h:512 * (h + 1)],
                            start=(i == 0), stop=(i == 1))
                act_set = (2, 5, 7) if nsl == 4 else (1,)
                for h in range(nsl):
                    dst = ot[:, w * j + 512 * h:w * j + 512 * (h + 1)]
                    if ci in act_set:     # 3 of 8 copies on ACT, rest on DVE
                        nc.scalar.activation(dst, pss[h], AF.Identity,
                                             bias=bcol[(s, j)],
                                             scale=ccol[(s, j)])
                    else:
                        nc.vector.tensor_scalar(dst, pss[h], ccol[(s, j)],
                                                bcol[(s, j)],
                                                op0=OP.mult, op1=OP.add)
                    ci += 1
            if sfx == "c":   # final writes on the idle HWDGE rings (short tail)
                nc.sync.dma_start(o0_d[:, c0:c0 + w], ot[:, 0:w])
                nc.scalar.dma_start(o1_d[:, c0:c0 + w], ot[:, w:2 * w])
            else:
                nc.gpsimd.dma_start(o0_d[:, c0:c0 + w], ot[:, 0:w])
                nc.scalar.dma_start(o1_d[:, c0:c0 + w], ot[:, w:2 * w])

    nc.finalize()
    return nc


def prep_host_inputs(ctx, x, W_layer, b_layer, W_bias, W_gate, b_gate, W_kv,
                     rows=ROWS):
    """Build the per-core in_maps (host-side sharding + constant re-layout)."""
    import ml_dtypes

    bf16 = ml_dtypes.bfloat16
    ctx = np.asarray(ctx, np.float32)
    x = np.asarray(x, np.float32)
    W_layer = np.asarray(W_layer, np.float32)
    b_layer = np.asarray(b_layer, np.float32)
    W_bias = np.asarray(W_bias, np.float32)
    W_gate = np.asarray(W_gate, np.float32)
    b_gate = np.asarray(b_gate, np.float32)
    W_kv = np.asarray(W_kv, np.float32)

    wcatT = np.zeros((256, 768), np.float32)
    wcatT[:DCTX, 0:256] = W_gate.T
    wcatT[:DCTX, 256:512] = W_bias.T
    wcatT[:DCTX, 512:768] = W_kv.T
    WlT = W_layer.T  # [din, o]

    rowpack = np.zeros((1, 1024), np.float32)
    rowpack[0, _RP_BLR:_RP_BLR + 256] = b_layer
    rowpack[0, _RP_BGR:_RP_BGR + 256] = b_gate
    rowpack[0, _RP_ONES:_RP_ONES + 512] = 1.0

    cold = np.zeros((128, _COLD_COLS), np.float32)
    cold[:, _PK_WL0:_PK_WL0 + 256] = W_layer[0:128]
    cold[:, _PK_WL1:_PK_WL1 + 256] = W_layer[128:256]
    cold[:, _PK_WLT0:_PK_WLT0 + 256] = WlT[0:128]
    cold[:, _PK_WLT1:_PK_WLT1 + 256] = WlT[128:256]
    cold[:, _PK_BLC0:_PK_BLC0 + 1] = b_layer[0:128, None]
    cold[:, _PK_BLC1:_PK_BLC1 + 1] = b_layer[128:256, None]
    cold[:, _PK_ONESC:_PK_ONESC + 1] = 1.0

    base_hot = np.zeros((128, _HOT_COLS), np.float32)
    base_hot[:, _PK_WCAT0:_PK_WCAT0 + 768] = wcatT[0:128]
    base_hot[:, _PK_WCAT1:_PK_WCAT1 + 768] = wcatT[128:256]

    in_maps = []
    for c in range(NCORES):
        hot = base_hot.copy()
        for k in range(SPC):
            # ctx has DCTX=131 rows: split across the two 128-row halves
            cv = np.pad(ctx[SPC * c + k, 0], (0, 256 - DCTX))
            hot[0:128, _PK_CTXT0 + k] = cv[0:128]
            hot[0:128, _PK_CTXT1 + k] = cv[128:256]
        xT = x[SPC * c:SPC * (c + 1)].reshape(rows, DIN).T.astype(bf16)
        xTall = np.concatenate([xT[0:128], xT[128:256]], axis=1)
        in_maps.append({"xT": xTall, "packhot": hot, "packcold": cold,
                        "rowpack": rowpack})
    return in_maps


def unshard(results):
    """results[c] has outT0/outT1 [128, ROWS] bf16 -> out [B, N, DOUT] f32."""
    out = np.empty((B, N, DOUT), np.float32)
    for c in range(NCORES):
        oT = np.concatenate([np.asarray(results[c]["outT0"]),
                             np.asarray(results[c]["outT1"])], axis=0)
        out[SPC * c:SPC * (c + 1)] = \
            oT.T.astype(np.float32).reshape(SPC, N, DOUT)
    return out


def kernel(ctx, x, W_layer, b_layer, W_bias, W_gate, b_gate, W_kv):
    from concourse.bass_utils import run_bass_kernel_spmd

    nc = build_nc(ROWS)
    in_maps = prep_host_inputs(ctx, x, W_layer, b_layer, W_bias, W_gate,
                               b_gate, W_kv)
    res = run_bass_kernel_spmd(nc, in_maps, core_ids=list(range(NCORES)))
    return unshard(res.results)


# revision 29
# speedup vs baseline: 1.0065x; 1.0065x over previous
"""Trainium2 Bass kernel for nn_ConcatSquashLinearSA.

Math (per sample b, S=1):
    gate = sigmoid(ctx @ Wg.T + bg)          [256]
    bias = ctx @ Wb.T                        [256]
    kv   = ctx @ Wkv.T                       [256]
    E    = outer(kv, kv)                     [256,256]
    A    = softmax_rows(E)
    att  = A / (1e-9 + colsum(A))
    out  = (x @ Wl.T + bl) @ (I + att) * gate + bias

which folds to a single big matmul per sample:
    P'     = A / colsum(A)                  (gate factored out)
    W_raw  = Wl.T @ P' + Wl.T               [256,256]  (tiny, on-device, f32)
    b_fin  = (bl + bl @ P') * gate + bias   [256]
    out    = (x @ W_raw) * gate + b_fin     [16384,256] (the only big op)

Sharding: data-parallel over batch, 2 samples per core across 8 cores.

The big op is memory-bound, so all big I/O is bf16 (2e-2 tolerance; bf16
end-to-end emulated rel-err is 2.7e-3). The host pre-transposes x to
x^T [256, rows] bf16; the device computes out^T = (W_raw^T @ x^T) * gate
+ b_fin with e on partitions (gate/b_fin are per-partition scalars of the
PSUM->SBUF copy), and the host transposes back. This removes all
on-device transposes and halves HBM traffic vs f32.

Schedule notes (from perfetto/NTFF traces):
  - every dma_start costs ~0.65us of issuing-engine time: constants are
    packed into 2 DMAs, reads all go on the otherwise-idle sync ring
  - softmax/W_raw preamble is a serial cross-engine chain (~1us/hop):
    exp+rowsum fused via accum_out, colsum taken directly from expE with
    lhsT=rowrecip, gate applied at copy time (not in the preamble)
  - reads run ~12 chunks ahead (xin bufs) so HBM streams during preamble
  - psum tiles span 2 banks -> half the copies/semaphores; copies split
    3:1 between DVE and ACT (gpsimd has no PSUM port)
"""

import numpy as np

B, N, DIN, DOUT, DCTX = 16, 16384, 256, 256, 131
NCORES = 8
SPC = B // NCORES           # samples per core
ROWS = SPC * N              # x rows per core (columns of x^T)
CH = 2048                   # x^T columns per macro-chunk

# column offsets in the packed constant blocks (hot: needed by first MMs)
_PK_CTXT0, _PK_CTXT1 = 0, 2
_PK_WCAT0, _PK_WCAT1 = 4, 772
_HOT_COLS = 1540
_PK_WL0, _PK_WL1 = 0, 256
_PK_WLT0, _PK_WLT1 = 512, 768
_PK_BLC0, _PK_BLC1, _PK_ONESC = 1024, 1025, 1026
_COLD_COLS = 1027
# row pack [1, 1024]: blr | bgr | ones512
_RP_BLR, _RP_BGR, _RP_ONES = 0, 256, 512


def build_nc(rows=ROWS):
    import concourse.bass as bass
    import concourse.tile as tile
    from concourse import bacc, mybir
    from contextlib import ExitStack

    f32 = mybir.dt.float32
    bf16 = mybir.dt.bfloat16
    AF = mybir.ActivationFunctionType
    AX = mybir.AxisListType
    OP = mybir.AluOpType

    n_chunks = rows // CH

    nc = bacc.Bacc()
    x_d = nc.declare_dram_parameter("xT", [128, 2 * rows], bf16, isOutput=False)
    hot_d = nc.declare_dram_parameter("packhot", [128, _HOT_COLS], f32,
                                      isOutput=False)
    cold_d = nc.declare_dram_parameter("packcold", [128, _COLD_COLS], f32,
                                       isOutput=False)
    rowp_d = nc.declare_dram_parameter("rowpack", [1, 1024], f32,
                                       isOutput=False)
    o0_d = nc.declare_dram_parameter("outT0", [128, rows], bf16, isOutput=True)
    o1_d = nc.declare_dram_parameter("outT1", [128, rows], bf16, isOutput=True)

    with tile.TileContext(nc) as tc, ExitStack() as ctx:
        consts = ctx.enter_context(tc.tile_pool(name="consts", bufs=1))
        spool = ctx.enter_context(tc.tile_pool(name="scratch", bufs=2))
        perm = ctx.enter_context(tc.tile_pool(name="persample", bufs=1))
        pps = ctx.enter_context(tc.tile_pool(name="pps", bufs=1, space="PSUM"))
        pout = ctx.enter_context(tc.tile_pool(name="pout", bufs=7, space="PSUM"))
        xin = ctx.enter_context(tc.tile_pool(name="xin", bufs=12))
        osb = ctx.enter_context(tc.tile_pool(name="osb", bufs=4))
        xlast = ctx.enter_context(tc.tile_pool(name="xlast", bufs=1))
        olast = ctx.enter_context(tc.tile_pool(name="olast", bufs=1))

        hot = consts.tile([128, _HOT_COLS], f32, name="packhot", tag="packhot")
        nc.sync.dma_start(hot, hot_d[:, :])
        rp = consts.tile([1, 1024], f32, name="rowpack", tag="rowpack")
        nc.sync.dma_start(rp, rowp_d[:, :])
        cold = consts.tile([128, _COLD_COLS], f32, name="packcold",
                           tag="packcold")
        nc.sync.dma_start(cold, cold_d[:, :])

        ctxT = [hot[:, _PK_CTXT0:_PK_CTXT0 + SPC], hot[:, _PK_CTXT1:_PK_CTXT1 + SPC]]
        wcat = [hot[:, _PK_WCAT0:_PK_WCAT0 + 768], hot[:, _PK_WCAT1:_PK_WCAT1 + 768]]
        wl = [cold[:, _PK_WL0:_PK_WL0 + 256], cold[:, _PK_WL1:_PK_WL1 + 256]]
        wlT = [cold[:, _PK_WLT0:_PK_WLT0 + 256], cold[:, _PK_WLT1:_PK_WLT1 + 256]]
        blc = [cold[:, _PK_BLC0:_PK_BLC0 + 1], cold[:, _PK_BLC1:_PK_BLC1 + 1]]
        onesc = cold[:, _PK_ONESC:_PK_ONESC + 1]
        blr = rp[0:1, _RP_BLR:_RP_BLR + 256]
        bgr = rp[0:1, _RP_BGR:_RP_BGR + 256]
        onesr = rp[0:1, _RP_ONES:_RP_ONES + 128]
        ones1 = rp[0:1, _RP_ONES:_RP_ONES + 1]

        # warm the ACT/DVE function tables before the real chain needs them
        warm = spool.tile([1, 4], f32, name="warm", tag="warm")
        nc.scalar.activation(warm[0:1, 0:1], ones1, AF.Exp)
        nc.scalar.activation(warm[0:1, 1:2], ones1, AF.Tanh)
        nc.scalar.activation(warm[0:1, 2:3], ones1, AF.Identity)
        nc.vector.reciprocal(warm[0:1, 3:4], ones1)

        weff = {}
        bcol = {}   # (s, j) -> [128,1] b_fin^T half (per-partition out bias)
        for s in range(SPC):
            # ---- ctx projections: [gate_pre | bias | kv] = ctx @ WcatT ----
            cat1 = pps.tile([1, 512], f32, name=f"cat1_{s}", tag="ps")
            nc.tensor.matmul(cat1, lhsT=ctxT[0][:, s:s + 1],
                             rhs=wcat[0][:, 0:512], start=True, stop=False)
            nc.tensor.matmul(cat1, lhsT=ctxT[1][:, s:s + 1],
                             rhs=wcat[1][:, 0:512], start=False, stop=True)
            cat2 = pps.tile([1, 256], f32, name=f"cat2_{s}", tag="ps")
            nc.tensor.matmul(cat2, lhsT=ctxT[0][:, s:s + 1],
                             rhs=wcat[0][:, 512:768], start=True, stop=False)
            nc.tensor.matmul(cat2, lhsT=ctxT[1][:, s:s + 1],
                             rhs=wcat[1][:, 512:768], start=False, stop=True)
            svec = spool.tile([1, 768], f32, name=f"svec{s}", tag="svec")
            nc.vector.tensor_copy(svec[:, 0:512], cat1)
            nc.vector.tensor_copy(svec[:, 512:768], cat2)

            # gate = sigmoid(pre) = 0.5*tanh(0.5*pre) + 0.5  (same ACT table)
            gpre = spool.tile([1, 256], f32, name=f"gpre{s}", tag="gpre")
            nc.vector.tensor_add(gpre, svec[:, 0:256], bgr)
            th = spool.tile([1, 256], f32, name=f"th{s}", tag="th")
            nc.scalar.activation(th, gpre, AF.Tanh, scale=0.5)
            gate = spool.tile([1, 256], f32, name=f"gate{s}", tag="gate")
            nc.vector.tensor_scalar(gate, th, 0.5, 0.5, op0=OP.mult, op1=OP.add)

            # ---- E = outer(kv, kv); fused exp+rowsum (|E|<=~8: exp safe) ----
            expEs, rcs_ = [], []
            for i in range(2):
                E = pps.tile([128, 256], f32, name=f"E{s}{i}", tag="ps")
                nc.tensor.matmul(E, lhsT=svec[0:1, 512 + 128 * i:640 + 128 * i],
                                 rhs=svec[0:1, 512:768], start=True, stop=True)
                expE = spool.tile([128, 256], f32, name=f"expE{s}{i}", tag="expE")
                sm = spool.tile([128, 1], f32, name=f"sm{s}{i}", tag="sm")
                nc.scalar.activation(expE, E, AF.Exp, accum_out=sm)
                rc = spool.tile([128, 1], f32, name=f"rc{s}{i}", tag="rc")
                nc.vector.reciprocal(rc, sm)
                expEs.append(expE)
                rcs_.append(rc)

            # colsum(A) directly from expE: cs_e = sum_d rc[d]*expE[d,e]
            cs = pps.tile([1, 256], f32, name=f"cs{s}", tag="ps")
            nc.tensor.matmul(cs, lhsT=rcs_[0], rhs=expEs[0], start=True, stop=False)
            nc.tensor.matmul(cs, lhsT=rcs_[1], rhs=expEs[1], start=False, stop=True)
            rcsum = spool.tile([1, 256], f32, name=f"rcsum{s}", tag="rcsum")
            nc.vector.reciprocal(rcsum, cs)   # colsum >= 0.8 on this data

            # broadcast 1/colsum and gate to [128,256]
            vbp = pps.tile([128, 256], f32, name=f"vbp{s}", tag="ps")
            nc.tensor.matmul(vbp, lhsT=onesr, rhs=rcsum, start=True, stop=True)
            Vb = spool.tile([128, 256], f32, name=f"Vb{s}", tag="Vb")
            nc.vector.tensor_copy(Vb, vbp)
            gbp = pps.tile([128, 256], f32, name=f"gbp{s}", tag="ps")
            nc.tensor.matmul(gbp, lhsT=onesr, rhs=gate, start=True, stop=True)
            GateB = spool.tile([128, 256], f32, name=f"GateB{s}", tag="GateB")
            nc.vector.tensor_copy(GateB, gbp)
            # P' = expE * rc (row) * 1/colsum (col)
            P = []
            for i in range(2):
                P1 = spool.tile([128, 256], f32, name=f"P1{s}{i}", tag="P1")
                nc.vector.tensor_scalar_mul(P1, expEs[i], rcs_[i])
                Pi = spool.tile([128, 256], f32, name=f"P{s}{i}", tag="P")
                nc.vector.tensor_mul(Pi, P1, Vb)
                P.append(Pi)

            # ---- W_eff = (Wl.T @ P' + Wl.T) * gateB; rows d-half i, bf16 ----
            # (gate folded into the weights so the psum copy is one ALU pass)
            for i in range(2):
                wp = pps.tile([128, 256], f32, name=f"wp{s}{i}", tag="ps")
                nc.tensor.matmul(wp, lhsT=wl[0][:, 128 * i:128 * (i + 1)], rhs=P[0],
                                 start=True, stop=False)
                nc.tensor.matmul(wp, lhsT=wl[1][:, 128 * i:128 * (i + 1)], rhs=P[1],
                                 start=False, stop=True)
                wpre = spool.tile([128, 256], f32, name=f"wpre{s}{i}", tag="wpre")
                nc.vector.tensor_add(wpre, wp, wlT[i])
                wsb = perm.tile([128, 256], bf16, name=f"weff{s}{i}",
                                tag=f"weff{s}{i}")
                nc.vector.tensor_mul(wsb, wpre, GateB)
                weff[(s, i)] = wsb

            # ---- b_fin = (bl + bl @ P') * gate + bias ----
            qp = pps.tile([1, 256], f32, name=f"qp{s}", tag="ps")
            nc.tensor.matmul(qp, lhsT=blc[0], rhs=P[0], start=True, stop=False)
            nc.tensor.matmul(qp, lhsT=blc[1], rhs=P[1], start=False, stop=True)
            tb2 = spool.tile([1, 256], f32, name=f"tb2{s}", tag="tb2")
            nc.vector.tensor_add(tb2, blr, qp)
            tb3 = spool.tile([1, 256], f32, name=f"tb3{s}", tag="tb3")
            nc.vector.tensor_mul(tb3, tb2, gate)
            bfin = spool.tile([1, 256], f32, name=f"bfin{s}", tag="bfin")
            nc.vector.tensor_add(bfin, tb3, svec[:, 256:512])

            # ---- per-partition copy bias: b_fin^T halves ----
            for j in range(2):
                bp = pps.tile([128, 1], f32, name=f"bp{s}{j}", tag="ps")
                nc.tensor.matmul(bp, lhsT=bfin[0:1, 128 * j:128 * (j + 1)],
                                 rhs=ones1, start=True, stop=True)
                bc = perm.tile([128, 1], f32, name=f"bc{s}{j}", tag=f"bc{s}{j}")
                nc.vector.tensor_copy(bc, bp)
                bcol[(s, j)] = bc

        # ---- main loop: out^T[e,n] = (sum_d W_raw[d,e] x^T[d,n])*gate[e]
        #      + b_fin[e];  e-half j on partitions, d contracted; bf16 streams.
        x3 = x_d.rearrange("p (i n) -> p i n", i=2)
        sched = [(CH * t, CH, "") for t in range(n_chunks - 1)]
        base = CH * (n_chunks - 1)
        sched += [(base, 1024, "a"), (base + 1024, 512, "b"),
                  (base + 1536, 512, "c")]
        for (c0, w, sfx) in sched:
            s = c0 // (rows // SPC)
            xpool, opool = (xin, osb) if not sfx else (xlast, olast)
            xt = xpool.tile([128, 2, w], bf16, name="xt" + sfx, tag="xt" + sfx)
            nc.sync.dma_start(xt, x3[:, :, c0:c0 + w])
            ot = opool.tile([128, 2 * w], bf16, name="ot" + sfx, tag="ot" + sfx)
            nsl = w // 512
            ci = 0
            for j in range(2):
                # one stationary weight per (i, j): nsl consecutive matmuls
                pss = [pout.tile([128, 512], f32, name="ps", tag="ops")
                       for _ in range(nsl)]
                for i in range(2):
                    for h in range(nsl):
                        nc.tensor.matmul(
                            pss[h],
                            lhsT=weff[(s, i)][:, 128 * j:128 * (j + 1)],
                            rhs=xt[:, i, 512 * h:512 * (h + 1)],
                            start=(i == 0), stop=(i == 1))
                act_set = (2, 5, 7) if nsl == 4 else (1,)
                for h in range(nsl):
                    dst = ot[:, w * j + 512 * h:w * j + 512 * (h + 1)]
                    if ci in act_set:     # 3 of 8 copies on ACT, rest on DVE
                        nc.scalar.activation(dst, pss[h], AF.Identity,
                                             bias=bcol[(s, j)])
                    else:
                        nc.vector.tensor_scalar_add(dst, pss[h], bcol[(s, j)])
                    ci += 1
            if sfx == "c":   # final writes on the idle HWDGE rings (short tail)
                nc.sync.dma_start(o0_d[:, c0:c0 + w], ot[:, 0:w])
                nc.scalar.dma_start(o1_d[:, c0:c0 + w], ot[:, w:2 * w])
            else:
                nc.gpsimd.dma_start(o0_d[:, c0:c0 + w], ot[:, 0:w])
                nc.scalar.dma_start(o1_d[:, c0:c0 + w], ot[:, w:2 * w])

    nc.finalize()
    return nc


def prep_host_inputs(ctx, x, W_layer, b_layer, W_bias, W_gate, b_gate, W_kv,
                     rows=ROWS):
    """Build the per-core in_maps (host-side sharding + constant re-layout)."""
    import ml_dtypes

    bf16 = ml_dtypes.bfloat16
    ctx = np.asarray(ctx, np.float32)
    x = np.asarray(x, np.float32)
    W_layer = np.asarray(W_layer, np.float32)
    b_layer = np.asarray(b_layer, np.float32)
    W_bias = np.asarray(W_bias, np.float32)
    W_gate = np.asarray(W_gate, np.float32)
    b_gate = np.asarray(b_gate, np.float32)
    W_kv = np.asarray(W_kv, np.float32)

    wcatT = np.zeros((256, 768), np.float32)
    wcatT[:DCTX, 0:256] = W_gate.T
    wcatT[:DCTX, 256:512] = W_bias.T
    wcatT[:DCTX, 512:768] = W_kv.T
    WlT = W_layer.T  # [din, o]

    rowpack = np.zeros((1, 1024), np.float32)
    rowpack[0, _RP_BLR:_RP_BLR + 256] = b_layer
    rowpack[0, _RP_BGR:_RP_BGR + 256] = b_gate
    rowpack[0, _RP_ONES:_RP_ONES + 512] = 1.0

    cold = np.zeros((128, _COLD_COLS), np.float32)
    cold[:, _PK_WL0:_PK_WL0 + 256] = W_layer[0:128]
    cold[:, _PK_WL1:_PK_WL1 + 256] = W_layer[128:256]
    cold[:, _PK_WLT0:_PK_WLT0 + 256] = WlT[0:128]
    cold[:, _PK_WLT1:_PK_WLT1 + 256] = WlT[128:256]
    cold[:, _PK_BLC0:_PK_BLC0 + 1] = b_layer[0:128, None]
    cold[:, _PK_BLC1:_PK_BLC1 + 1] = b_layer[128:256, None]
    cold[:, _PK_ONESC:_PK_ONESC + 1] = 1.0

    base_hot = np.zeros((128, _HOT_COLS), np.float32)
    base_hot[:, _PK_WCAT0:_PK_WCAT0 + 768] = wcatT[0:128]
    base_hot[:, _PK_WCAT1:_PK_WCAT1 + 768] = wcatT[128:256]

    in_maps = []
    for c in range(NCORES):
        hot = base_hot.copy()
        for k in range(SPC):
            # ctx has DCTX=131 rows: split across the two 128-row halves
            cv = np.pad(ctx[SPC * c + k, 0], (0, 256 - DCTX))
            hot[0:128, _PK_CTXT0 + k] = cv[0:128]
            hot[0:128, _PK_CTXT1 + k] = cv[128:256]
        xT = x[SPC * c:SPC * (c + 1)].reshape(rows, DIN).T.astype(bf16)
        xTall = np.concatenate([xT[0:128], xT[128:256]], axis=1)
        in_maps.append({"xT": xTall, "packhot": hot, "packcold": cold,
                        "rowpack": rowpack})
    return in_maps


def unshard(results):
    """results[c] has outT0/outT1 [128, ROWS] bf16 -> out [B, N, DOUT] f32."""
    out = np.empty((B, N, DOUT), np.float32)
    for c in range(NCORES):
        oT = np.concatenate([np.asarray(results[c]["outT0"]),
                             np.asarray(results[c]["outT1"])], axis=0)
        out[SPC * c:SPC * (c + 1)] = \
            oT.T.astype(np.float32).reshape(SPC, N, DOUT)
    return out


def kernel(ctx, x, W_layer, b_layer, W_bias, W_gate, b_gate, W_kv):
    from concourse.bass_utils import run_bass_kernel_spmd

    nc = build_nc(ROWS)
    in_maps = prep_host_inputs(ctx, x, W_layer, b_layer, W_bias, W_gate,
                               b_gate, W_kv)
    res = run_bass_kernel_spmd(nc, in_maps, core_ids=list(range(NCORES)))
    return unshard(res.results)
